# revision 20
# baseline (speedup 1.0000x reference)
"""Causal self-attention (B=4, T=2048, C=1024, H=16, D=64) on 8 TRN2 NeuronCores.

Sharding: 2D (batch x head-group). Core c handles batch b = c//2 and head
group g = c%2 (heads 8g..8g+7).  Host pre-transposes all inputs so the
device kernel needs no on-chip transposes:
  - xT  [C, T]    : x[b].T
  - wqT/wkT/wvT [C, 512] : w_qkv row-slices for this head group, transposed
  - woT [512, C]  : w_proj column-slice, transposed
Each core computes a partial projected output yT [C, T] for its batch
(contribution of its 8 heads); the host sums the two head-group partials
per batch and transposes back.

Device flow per core (all matmuls are PE `out = lhsT.T @ rhs`):
  1. QKV: qT/kT [512, T] (features on partitions) and v [T, 512] (natural)
     into DRAM scratch.
  2. Attention per head-pair hp (2 heads share the 128-partition dim):
     scores sT[k,q] via row-tiled concurrent matmuls (K=64 each head),
     exp on ScalarE (scale=1/8 fused), causal: skip above-diagonal k-tiles
     and mask diagonal tiles on VectorE, PV matmul accumulates oT [65, 512]
     with a ones-column in v giving the softmax denominator in row 64.
     Normalize: reciprocal (DVE) + ones-matmul partition-broadcast (PE).
  3. Proj: yT = woT.T @ o_pack accumulated over the 4 head pairs.
"""

import os
from contextlib import ExitStack

import numpy as np
import ml_dtypes

import concourse.bass as bass
import concourse.bacc as bacc
import concourse.mybir as mybir
import concourse.tile as tile
from concourse.bass_utils import run_bass_kernel_spmd, checkenv

B, T, C = 4, 2048, 1024
H, D = 16, 64
NCORES = 8
F = 512                    # qkv features per matrix per core (8 heads x 64)
CT = C // 128              # 8 contraction tiles
TT = T // 128              # 16 token tiles
QB = T // 512              # 4 query blocks of 512
HP = 4                     # head pairs per core

DTYPE_MODE = os.environ.get("KERNEL_DTYPE", "f32r")  # f32 | f32r | bf16

_F32 = mybir.dt.float32
_EXP = mybir.ActivationFunctionType.Exp

_cache = {}


def _build_nc():
    # fp32r == TF32 (10-bit mantissa) at full PE rate; the BIR verifier
    # requires every producer feeding an fp32r matmul to emit fp32r, so the
    # whole store path uses the dtype and the host pre-rounds its arrays.
    dt_store = {"f32": _F32, "f32r": mybir.dt.float32r,
                "bf16": mybir.dt.bfloat16}[DTYPE_MODE]

    def mm(ap):
        return ap

    nc = bacc.Bacc("TRN2", target_bir_lowering=False, debug=False,
                   num_devices=NCORES)

    xT = nc.dram_tensor("xT", [C, T], dt_store, kind="ExternalInput").ap()
    wqT = nc.dram_tensor("wqT", [C, F], dt_store, kind="ExternalInput").ap()
    wkT = nc.dram_tensor("wkT", [C, F], dt_store, kind="ExternalInput").ap()
    wvT = nc.dram_tensor("wvT", [C, F], dt_store, kind="ExternalInput").ap()
    woT = nc.dram_tensor("woT", [F, C], dt_store, kind="ExternalInput").ap()
    maskd = nc.dram_tensor("mask", [128, 4, 512], dt_store,
                           kind="ExternalInput").ap()
    onesd = nc.dram_tensor("ones", [128, 128], dt_store,
                           kind="ExternalInput").ap()
    yT = nc.dram_tensor("yT", [C, T], _F32, kind="ExternalOutput").ap()

    with tile.TileContext(nc) as tc, ExitStack() as top:
        dram = top.enter_context(tc.tile_pool(name="scratch", bufs=1,
                                              space="DRAM"))
        qkT_d = dram.tile([2, HP, 128, T], dt_store)   # [q/k, ftile, part, tok]
        v_d = dram.tile([TT, 128, F], dt_store)        # v natural, tok-tiled

        # ---------------- Phase 1: QKV projections ----------------
        with tc.tile_pool(name="wqkv", bufs=1) as wpool, \
             tc.tile_pool(name="xin", bufs=2) as xpool, \
             tc.tile_pool(name="qkv_st", bufs=4) as stpool, \
             tc.tile_pool(name="qkv_ps", bufs=4, space="PSUM") as pspool:
            wq_sb = wpool.tile([128, CT, F], dt_store)
            wk_sb = wpool.tile([128, CT, F], dt_store)
            wv_sb = wpool.tile([128, CT, F], dt_store)
            nc.sync.dma_start(out=wq_sb[:],
                              in_=wqT.rearrange("(ct p) f -> p ct f", p=128))
            nc.sync.dma_start(out=wk_sb[:],
                              in_=wkT.rearrange("(ct p) f -> p ct f", p=128))
            nc.sync.dma_start(out=wv_sb[:],
                              in_=wvT.rearrange("(ct p) f -> p ct f", p=128))

            xT_r = xT.rearrange("(ct p) t -> p ct t", p=128)
            TH = T // 2
            for th in range(2):  # token halves to bound SBUF
                x_sb = xpool.tile([128, CT, TH], dt_store)
                nc.sync.dma_start(out=x_sb[:],
                                  in_=xT_r[:, :, th * TH:(th + 1) * TH])

                def _store(ps, dst_ap):
                    # DMA cannot read PSUM: stage via ScalarE copy
                    st = stpool.tile([128, 512], dt_store, tag="st")
                    nc.scalar.copy(st[:], ps[:])
                    nc.sync.dma_start(out=dst_ap, in_=st[:])

                # qT / kT: [feat 128, tok 512] tiles
                for mt, w_sb in ((0, wq_sb), (1, wk_sb)):
                    for ft in range(4):
                        for tb in range(TH // 512):
                            ps = pspool.tile([128, 512], _F32, tag="ps")
                            for ct in range(CT):
                                nc.tensor.matmul(
                                    ps[:],
                                    mm(w_sb[:, ct, ft * 128:(ft + 1) * 128]),
                                    mm(x_sb[:, ct, tb * 512:(tb + 1) * 512]),
                                    start=(ct == 0), stop=(ct == CT - 1))
                            tok0 = th * TH + tb * 512
                            _store(ps, qkT_d[mt, ft, :, tok0:tok0 + 512])
                # v natural: [tok 128, feat 512] tiles
                for ttl in range(TH // 128):
                    ps = pspool.tile([128, 512], _F32, tag="ps")
                    for ct in range(CT):
                        nc.tensor.matmul(
                            ps[:],
                            mm(x_sb[:, ct, ttl * 128:(ttl + 1) * 128]),
                            mm(wv_sb[:, ct, :]),
                            start=(ct == 0), stop=(ct == CT - 1))
                    _store(ps, v_d[th * (TH // 128) + ttl])

        # ---------------- Phase 2+3 shared: o_pack ----------------
        with tc.tile_pool(name="opack", bufs=1) as opool:
            o_pack = opool.tile([128, HP, T], dt_store)

            v_r = v_d.rearrange("tt p (h d) -> p tt h d", h=8)
            with tc.tile_pool(name="qk", bufs=2) as qkpool, \
                 tc.tile_pool(name="vv", bufs=2) as vpool, \
                 tc.tile_pool(name="msk", bufs=1) as mpool, \
                 tc.tile_pool(name="exp", bufs=6) as epool, \
                 tc.tile_pool(name="osb", bufs=3) as osbpool, \
                 tc.tile_pool(name="rc", bufs=2) as rcpool, \
                 tc.tile_pool(name="ones", bufs=1) as onepool, \
                 tc.tile_pool(name="sc_ps", bufs=4, space="PSUM") as scps, \
                 tc.tile_pool(name="o_ps", bufs=2, space="PSUM") as ops, \
                 tc.tile_pool(name="bc_ps", bufs=2, space="PSUM") as bcps:

                mask_sb = mpool.tile([128, 4, 512], dt_store)
                nc.sync.dma_start(out=mask_sb[:], in_=maskd[:])
                ones_sb = onepool.tile([128, 128], dt_store)
                nc.sync.dma_start(out=ones_sb[:], in_=onesd)

                for hp in range(HP):
                    q_sb = qkpool.tile([128, T], dt_store, tag="q")
                    k_sb = qkpool.tile([128, T], dt_store, tag="k")
                    v_sb = vpool.tile([128, TT, 2, D + 1], dt_store)
                    nc.sync.dma_start(out=q_sb[:], in_=qkT_d[0, hp])
                    nc.sync.dma_start(out=k_sb[:], in_=qkT_d[1, hp])
                    for hl in range(2):  # 3-dim AP limit per DMA
                        nc.sync.dma_start(out=v_sb[:, :, hl, 0:D],
                                          in_=v_r[:, :, 2 * hp + hl, :])
                    # ones column for the softmax row-sum (memset can't
                    # write f32r -> DMA from host ones)
                    nc.sync.dma_start(out=v_sb[:, :, :, D:D + 1],
                                      in_=onesd[:, 0:32])

                    for qb in range(QB):
                        kts = 4 * (qb + 1)
                        oA = ops.tile([D + 1, 512], _F32, tag="o")
                        oB = ops.tile([D + 1, 512], _F32, tag="o")
                        qsl = slice(qb * 512, (qb + 1) * 512)

                        pend = []  # software pipeline: scores(kt) ahead of PV(kt-1)

                        def _pv(kt, eA, eB):
                            nc.tensor.matmul(
                                oA[:], mm(v_sb[:, kt, 0, :]), mm(eA[:]),
                                start=(kt == 0), stop=(kt == kts - 1))
                            nc.tensor.matmul(
                                oB[:], mm(v_sb[:, kt, 1, :]), mm(eB[:]),
                                start=(kt == 0), stop=(kt == kts - 1))

                        for kt in range(kts):
                            psA = scps.tile([128, 512], _F32, tag="s")
                            psB = scps.tile([128, 512], _F32, tag="s")
                            ksl = slice(kt * 128, (kt + 1) * 128)
                            nc.tensor.matmul(psA[:], mm(k_sb[0:64, ksl]),
                                             mm(q_sb[0:64, qsl]),
                                             start=True, stop=True,
                                             tile_position=(0, 0))
                            nc.tensor.matmul(psB[:], mm(k_sb[64:128, ksl]),
                                             mm(q_sb[64:128, qsl]),
                                             start=True, stop=True,
                                             tile_position=(64, 0))
                            eA = epool.tile([128, 512], dt_store, tag="e")
                            eB = epool.tile([128, 512], dt_store, tag="e")
                            nc.scalar.activation(eA[:], psA[:], _EXP, scale=0.125)
                            nc.scalar.activation(eB[:], psB[:], _EXP, scale=0.125)
                            rel = kt - 4 * qb
                            if rel >= 0:  # diagonal tile: causal mask
                                nc.vector.tensor_mul(eA[:], eA[:],
                                                     mask_sb[:, rel, :])
                                nc.vector.tensor_mul(eB[:], eB[:],
                                                     mask_sb[:, rel, :])
                            pend.append((kt, eA, eB))
                            if len(pend) > 1:
                                _pv(*pend.pop(0))
                        _pv(*pend.pop(0))

                        # normalize: o[d, q] / r[q], r in row 64 of psum
                        rc = rcpool.tile([128, 1024], _F32)
                        nc.vector.reciprocal(rc[64:65, 0:512], oA[64:65, :])
                        nc.vector.reciprocal(rc[64:65, 512:1024], oB[64:65, :])
                        if dt_store == _F32:
                            rcx = rc
                        else:
                            rcx = rcpool.tile([128, 1024], dt_store, tag="rcx")
                            nc.vector.tensor_copy(rcx[64:65, :], rc[64:65, :])
                        # broadcast 1/r along partitions via ones-matmul
                        # (MM dst must start at psum partition 0 -> two
                        # full-width tiles, one per head)
                        bcA = bcps.tile([128, 512], _F32, tag="bc")
                        bcB = bcps.tile([128, 512], _F32, tag="bc")
                        nc.tensor.matmul(bcA[:], mm(ones_sb[64:65, 0:128]),
                                         mm(rcx[64:65, 0:512]),
                                         start=True, stop=True,
                                         tile_position=(64, 0))
                        nc.tensor.matmul(bcB[:], mm(ones_sb[64:65, 0:128]),
                                         mm(rcx[64:65, 512:1024]),
                                         start=True, stop=True,
                                         tile_position=(64, 0))
                        # heads live at psum partitions 0-64; head B must land
                        # at o_pack partitions 64-127 -> sbuf->sbuf DMA shift
                        tmp = osbpool.tile([64, 1024], _F32, tag="t")
                        nc.scalar.copy(tmp[0:64, 0:512], oA[0:64, :])
                        nc.scalar.copy(tmp[0:64, 512:1024], oB[0:64, :])
                        ob2 = osbpool.tile([128, 512], _F32, tag="b")
                        nc.sync.dma_start(out=ob2[64:128, :],
                                          in_=tmp[0:64, 512:1024])
                        nc.vector.tensor_mul(o_pack[0:64, hp, qsl],
                                             tmp[0:64, 0:512], bcA[0:64, :])
                        nc.vector.tensor_mul(o_pack[64:128, hp, qsl],
                                             ob2[64:128, :], bcB[64:128, :])

            # ---------------- Phase 3: output projection ----------------
            with tc.tile_pool(name="wo", bufs=1) as wopool, \
                 tc.tile_pool(name="y_st", bufs=4) as ystpool, \
                 tc.tile_pool(name="y_ps", bufs=4, space="PSUM") as yps:
                wo_sb = wopool.tile([128, HP, C], dt_store)
                nc.sync.dma_start(out=wo_sb[:],
                                  in_=woT.rearrange("(hp p) o -> p hp o", p=128))
                for ot in range(C // 128):
                    for tb in range(QB):
                        ps = yps.tile([128, 512], _F32, tag="y")
                        for hp in range(HP):
                            nc.tensor.matmul(
                                ps[:],
                                mm(wo_sb[:, hp, ot * 128:(ot + 1) * 128]),
                                mm(o_pack[:, hp, tb * 512:(tb + 1) * 512]),
                                start=(hp == 0), stop=(hp == HP - 1))
                        yst = ystpool.tile([128, 512], _F32, tag="yst")
                        nc.scalar.copy(yst[:], ps[:])
                        nc.sync.dma_start(
                            out=yT[ot * 128:(ot + 1) * 128,
                                   tb * 512:(tb + 1) * 512],
                            in_=yst[:])
    nc.compile()  # bacc passes: split >1-wait instructions (TRN2 ISA limit)
    return nc


def _np_store():
    return np.float32 if DTYPE_MODE in ("f32", "f32r") else ml_dtypes.bfloat16


def _round_tf32(a):
    """Round-to-nearest-even onto the TF32 (10-bit mantissa) grid."""
    u = np.ascontiguousarray(a, dtype=np.float32).view(np.uint32)
    r = (u + 0x0FFF + ((u >> 13) & 1)) & np.uint32(0xFFFFE000)
    return r.view(np.float32)


def _prep(a):
    a = np.ascontiguousarray(a, dtype=np.float32)
    if DTYPE_MODE == "f32r":
        return _round_tf32(a)
    return a.astype(_np_store())


def _make_mask():
    kk = np.arange(128)[:, None, None]
    rr = np.arange(4)[None, :, None]
    qq = np.arange(512)[None, None, :]
    return ((rr * 128 + kk) <= qq).astype(_np_store())


LAST_RESULTS = None


def kernel(x, w_qkv, w_proj):
    global LAST_RESULTS
    if "nc" not in _cache:
        _cache["nc"] = _build_nc()
    nc = _cache["nc"]

    mask = _make_mask()
    x = np.asarray(x, dtype=np.float32).reshape(B, T, C)
    w_qkv = np.asarray(w_qkv, dtype=np.float32)
    w_proj = np.asarray(w_proj, dtype=np.float32)

    in_maps = []
    for core in range(NCORES):
        b, g = core // 2, core % 2
        fsl = slice(g * F, (g + 1) * F)
        in_maps.append({
            "xT": _prep(x[b].T),
            "wqT": _prep(w_qkv[0 * C:1 * C][fsl].T),
            "wkT": _prep(w_qkv[1 * C:2 * C][fsl].T),
            "wvT": _prep(w_qkv[2 * C:3 * C][fsl].T),
            "woT": _prep(w_proj[:, fsl].T),
            "mask": mask,
            "ones": np.ones((128, 128), np.float32) if DTYPE_MODE != "bf16"
                    else np.ones((128, 128), ml_dtypes.bfloat16),
        })

    LAST_RESULTS = run_bass_kernel_spmd(
        nc, in_maps, list(range(NCORES)), trace=checkenv("BASS_TRACE"))

    y = np.zeros((B, T, C), np.float32)
    for core in range(NCORES):
        b = core // 2
        y[b] += LAST_RESULTS.results[core]["yT"].T
    return y


# revision 28
# speedup vs baseline: 1.0323x; 1.0323x over previous
"""Causal self-attention (B=4, T=2048, C=1024, H=16, D=64) on 8 TRN2 NeuronCores.

Sharding: 2D (batch x head-group). Core c handles batch b = c//2 and head
group g = c%2 (heads 8g..8g+7).  Host pre-transposes all inputs so the
device kernel needs no on-chip transposes:
  - xT  [C, T]    : x[b].T
  - wqT/wkT/wvT [C, 512] : w_qkv row-slices for this head group, transposed
  - woT [512, C]  : w_proj column-slice, transposed
Each core computes a partial projected output yT [C, T] for its batch
(contribution of its 8 heads); the host sums the two head-group partials
per batch and transposes back.

Device flow per core (all matmuls are PE `out = lhsT.T @ rhs`, fp32r/TF32):
  A. v = x @ wv^T in natural [tok, feat] layout, resident in SBUF with a
     ones column appended per head (softmax denominator comes free from
     the PV matmul's row 64).
  B. Per head-pair hp: project qT/kT for just these 128 features, then
     attention: scores sT[k,q] via row-tiled concurrent matmuls (two heads
     share the 128-partition dim, K=64 each), exp on ScalarE (1/8 scale
     fused), causal = skip above-diagonal k-tiles + mask diagonal tiles on
     VectorE, PV matmul accumulates oT [65, 512].  The q/k projection of
     hp+1 gives the PE independent work while ACT runs hp's exps (keeps
     the HAM clock-gate at 2.4 GHz).  Normalization: 1/r via exp(-ln(r))
     on ScalarE, partition-broadcast via ones-matmul, multiply on DVE.
  C. yT = woT.T @ o_pack accumulated over the 4 head pairs.
"""

import os
from contextlib import ExitStack

import numpy as np
import ml_dtypes

import concourse.bass as bass
import concourse.bacc as bacc
import concourse.mybir as mybir
import concourse.tile as tile
from concourse.bass_utils import run_bass_kernel_spmd, checkenv

B, T, C = 4, 2048, 1024
H, D = 16, 64
NCORES = 8
F = 512                    # qkv features per matrix per core (8 heads x 64)
CT = C // 128              # 8 contraction tiles
TT = T // 128              # 16 token tiles
QB = T // 512              # 4 query blocks of 512
HP = 4                     # head pairs per core

DTYPE_MODE = os.environ.get("KERNEL_DTYPE", "f32r")  # f32 | f32r | bf16

_F32 = mybir.dt.float32
_EXP = mybir.ActivationFunctionType.Exp
_LN = mybir.ActivationFunctionType.Ln

_cache = {}


def _build_nc():
    dt_store = {"f32": _F32, "f32r": mybir.dt.float32r,
                "bf16": mybir.dt.bfloat16}[DTYPE_MODE]

    nc = bacc.Bacc("TRN2", target_bir_lowering=False, debug=False,
                   num_devices=NCORES)

    xT = nc.dram_tensor("xT", [C, T], dt_store, kind="ExternalInput").ap()
    wqT = nc.dram_tensor("wqT", [C, F], dt_store, kind="ExternalInput").ap()
    wkT = nc.dram_tensor("wkT", [C, F], dt_store, kind="ExternalInput").ap()
    wvT = nc.dram_tensor("wvT", [C, F], dt_store, kind="ExternalInput").ap()
    woT = nc.dram_tensor("woT", [F, C], dt_store, kind="ExternalInput").ap()
    maskd = nc.dram_tensor("mask", [128, 4, 512], dt_store,
                           kind="ExternalInput").ap()
    onesd = nc.dram_tensor("ones", [128, 128], dt_store,
                           kind="ExternalInput").ap()
    yT = nc.dram_tensor("yT", [C, T], _F32, kind="ExternalOutput").ap()

    xT_r = xT.rearrange("(ct p) t -> p ct t", p=128)

    with tile.TileContext(nc) as tc, ExitStack() as top:
        opool = top.enter_context(tc.tile_pool(name="opack", bufs=1))
        onepool = top.enter_context(tc.tile_pool(name="ones", bufs=1))
        ab = top.enter_context(ExitStack())  # pools freed before phase C
        xpool = ab.enter_context(tc.tile_pool(name="xin", bufs=2))
        wqk_pool = ab.enter_context(tc.tile_pool(name="wqk", bufs=1))
        vpool = ab.enter_context(tc.tile_pool(name="vfull", bufs=1))
        mpool = ab.enter_context(tc.tile_pool(name="msk", bufs=1))
        qkpool = ab.enter_context(tc.tile_pool(name="qk", bufs=2))
        epool = ab.enter_context(tc.tile_pool(name="exp", bufs=4))
        osbpool = ab.enter_context(tc.tile_pool(name="osb", bufs=1))
        rcpool = ab.enter_context(tc.tile_pool(name="rc", bufs=1))
        qkvps = ab.enter_context(tc.tile_pool(name="qkv_ps", bufs=2,
                                              space="PSUM"))

        wq_sb = wqk_pool.tile([128, CT, F], dt_store)
        wk_sb = wqk_pool.tile([128, CT, F], dt_store)
        nc.sync.dma_start(out=wq_sb[:],
                          in_=wqT.rearrange("(ct p) f -> p ct f", p=128))
        nc.sync.dma_start(out=wk_sb[:],
                          in_=wkT.rearrange("(ct p) f -> p ct f", p=128))
        mask_sb = mpool.tile([128, 4, 512], dt_store)
        nc.sync.dma_start(out=mask_sb[:], in_=maskd[:])
        ones_sb = onepool.tile([128, 128], dt_store)
        nc.sync.dma_start(out=ones_sb[:], in_=onesd)

        v_full = vpool.tile([128, TT, 8, D + 1], dt_store)
        nc.sync.dma_start(out=v_full[:, :, :, D:D + 1], in_=onesd[:, 0:128])
        o_pack = opool.tile([128, HP, T], dt_store)

        # ---------------- Phase A: v projection (natural layout) --------
        with tc.tile_pool(name="wv", bufs=1) as wvpool:
            wv_sb = wvpool.tile([128, CT, F], dt_store)
            nc.sync.dma_start(out=wv_sb[:],
                              in_=wvT.rearrange("(ct p) f -> p ct f", p=128))
            for tq in range(4):
                x_q = xpool.tile([128, CT, 512], dt_store, tag="x")
                nc.sync.dma_start(out=x_q[:],
                                  in_=xT_r[:, :, tq * 512:(tq + 1) * 512])
                for tl in range(4):
                    ps = qkvps.tile([128, 512], _F32, tag="ps")
                    for ct in range(CT):
                        nc.tensor.matmul(
                            ps[:], x_q[:, ct, tl * 128:(tl + 1) * 128],
                            wv_sb[:, ct, :],
                            start=(ct == 0), stop=(ct == CT - 1))
                    nc.vector.tensor_copy(
                        v_full[:, tq * 4 + tl, :, 0:D],
                        ps[:].rearrange("p (h d) -> p h d", h=8))

        # ---------- Phase B: per head-pair q/k projection + attention ----
        for hp in range(HP):
            fsl = slice(hp * 128, (hp + 1) * 128)
            q_sb = qkpool.tile([128, T], dt_store, tag="q")
            k_sb = qkpool.tile([128, T], dt_store, tag="k")
            for tq in range(4):
                x_q = xpool.tile([128, CT, 512], dt_store, tag="x")
                nc.sync.dma_start(out=x_q[:],
                                  in_=xT_r[:, :, tq * 512:(tq + 1) * 512])
                tsl = slice(tq * 512, (tq + 1) * 512)
                psq = qkvps.tile([128, 512], _F32, tag="ps")
                psk = qkvps.tile([128, 512], _F32, tag="ps")
                for ct in range(CT):
                    nc.tensor.matmul(psq[:], wq_sb[:, ct, fsl],
                                     x_q[:, ct, :],
                                     start=(ct == 0), stop=(ct == CT - 1))
                for ct in range(CT):
                    nc.tensor.matmul(psk[:], wk_sb[:, ct, fsl],
                                     x_q[:, ct, :],
                                     start=(ct == 0), stop=(ct == CT - 1))
                nc.vector.tensor_copy(q_sb[:, tsl], psq[:])
                nc.vector.tensor_copy(k_sb[:, tsl], psk[:])

            with tc.tile_pool(name=f"at{hp}_sc", bufs=4, space="PSUM") as scps, \
                 tc.tile_pool(name=f"at{hp}_o", bufs=2, space="PSUM") as ops:
                for qb in range(QB):
                    kts = 4 * (qb + 1)
                    oA = ops.tile([D + 1, 512], _F32, tag="o")
                    oB = ops.tile([D + 1, 512], _F32, tag="o")
                    qsl = slice(qb * 512, (qb + 1) * 512)

                    pend = []  # software pipeline: scores(kt) ahead of PV(kt-1)

                    def _pv(kt, eA, eB):
                        nc.tensor.matmul(
                            oA[:], v_full[:, kt, 2 * hp, :], eA[:],
                            start=(kt == 0), stop=(kt == kts - 1))
                        nc.tensor.matmul(
                            oB[:], v_full[:, kt, 2 * hp + 1, :], eB[:],
                            start=(kt == 0), stop=(kt == kts - 1))

                    for kt in range(kts):
                        psA = scps.tile([128, 512], _F32, tag="s")
                        psB = scps.tile([128, 512], _F32, tag="s")
                        ksl = slice(kt * 128, (kt + 1) * 128)
                        nc.tensor.matmul(psA[:], k_sb[0:64, ksl],
                                         q_sb[0:64, qsl],
                                         start=True, stop=True,
                                         tile_position=(0, 0))
                        nc.tensor.matmul(psB[:], k_sb[64:128, ksl],
                                         q_sb[64:128, qsl],
                                         start=True, stop=True,
                                         tile_position=(64, 0))
                        eA = epool.tile([128, 512], dt_store, tag="e")
                        eB = epool.tile([128, 512], dt_store, tag="e")
                        nc.scalar.activation(eA[:], psA[:], _EXP, scale=0.125)
                        nc.scalar.activation(eB[:], psB[:], _EXP, scale=0.125)
                        rel = kt - 4 * qb
                        if rel >= 0:  # diagonal tile: causal mask
                            nc.vector.tensor_mul(eA[:], eA[:],
                                                 mask_sb[:, rel, :])
                            nc.vector.tensor_mul(eB[:], eB[:],
                                                 mask_sb[:, rel, :])
                        pend.append((kt, eA, eB))
                        if len(pend) > 1:
                            _pv(*pend.pop(0))
                    _pv(*pend.pop(0))

                    # normalize: o[d, q] * (1/r)[q];  1/r = exp(-ln(r)) on
                    # ScalarE (DVE reciprocal on a 1-partition row is ~6x
                    # slower), then partition-broadcast via ones-matmul.
                    rc = rcpool.tile([128, 1024], _F32, tag="rc")
                    rcx = rcpool.tile([128, 1024], dt_store, tag="rcx")
                    nc.scalar.activation(rc[64:65, 0:512], oA[64:65, :], _LN)
                    nc.scalar.activation(rc[64:65, 512:1024], oB[64:65, :], _LN)
                    nc.scalar.activation(rcx[64:65, 0:512],
                                         rc[64:65, 0:512], _EXP, scale=-1.0)
                    nc.scalar.activation(rcx[64:65, 512:1024],
                                         rc[64:65, 512:1024], _EXP, scale=-1.0)
                    bcA = scps.tile([128, 512], _F32, tag="s")
                    bcB = scps.tile([128, 512], _F32, tag="s")
                    nc.tensor.matmul(bcA[:], ones_sb[64:65, 0:128],
                                     rcx[64:65, 0:512],
                                     start=True, stop=True,
                                     tile_position=(64, 0))
                    nc.tensor.matmul(bcB[:], ones_sb[64:65, 0:128],
                                     rcx[64:65, 512:1024],
                                     start=True, stop=True,
                                     tile_position=(64, 0))
                    # heads live at psum partitions 0-64; head B must land
                    # at o_pack partitions 64-127 -> sbuf->sbuf DMA shift
                    tmp = osbpool.tile([64, 1024], _F32, tag="t")
                    nc.vector.tensor_copy(tmp[0:64, 0:512], oA[0:64, :])
                    nc.vector.tensor_copy(tmp[0:64, 512:1024], oB[0:64, :])
                    ob2 = osbpool.tile([128, 512], _F32, tag="b")
                    nc.sync.dma_start(out=ob2[64:128, :],
                                      in_=tmp[0:64, 512:1024])
                    nc.vector.tensor_mul(o_pack[0:64, hp, qsl],
                                         tmp[0:64, 0:512], bcA[0:64, :])
                    nc.vector.tensor_mul(o_pack[64:128, hp, qsl],
                                         ob2[64:128, :], bcB[64:128, :])

        ab.close()  # free A/B pools (x, weights, v, qk, exp, ...) for phase C

        # ---------------- Phase C: output projection ----------------
        with tc.tile_pool(name="wo", bufs=1) as wopool, \
             tc.tile_pool(name="y_st", bufs=4) as ystpool, \
             tc.tile_pool(name="y_ps", bufs=4, space="PSUM") as yps:
            wo_sb = wopool.tile([128, HP, C], dt_store)
            nc.sync.dma_start(out=wo_sb[:],
                              in_=woT.rearrange("(hp p) o -> p hp o", p=128))
            for ot in range(C // 128):
                for tb in range(QB):
                    ps = yps.tile([128, 512], _F32, tag="y")
                    for hp in range(HP):
                        nc.tensor.matmul(
                            ps[:], wo_sb[:, hp, ot * 128:(ot + 1) * 128],
                            o_pack[:, hp, tb * 512:(tb + 1) * 512],
                            start=(hp == 0), stop=(hp == HP - 1))
                    yst = ystpool.tile([128, 512], _F32, tag="yst")
                    nc.vector.tensor_copy(yst[:], ps[:])
                    nc.sync.dma_start(
                        out=yT[ot * 128:(ot + 1) * 128,
                               tb * 512:(tb + 1) * 512],
                        in_=yst[:])
    nc.compile()  # bacc passes: split >1-wait instructions (TRN2 ISA limit)
    return nc


def _np_store():
    return np.float32 if DTYPE_MODE in ("f32", "f32r") else ml_dtypes.bfloat16


def _round_tf32(a):
    """Round-to-nearest-even onto the TF32 (10-bit mantissa) grid."""
    u = np.ascontiguousarray(a, dtype=np.float32).view(np.uint32)
    r = (u + 0x0FFF + ((u >> 13) & 1)) & np.uint32(0xFFFFE000)
    return r.view(np.float32)


def _prep(a):
    a = np.ascontiguousarray(a, dtype=np.float32)
    if DTYPE_MODE == "f32r":
        return _round_tf32(a)
    return a.astype(_np_store())


def _make_mask():
    kk = np.arange(128)[:, None, None]
    rr = np.arange(4)[None, :, None]
    qq = np.arange(512)[None, None, :]
    return ((rr * 128 + kk) <= qq).astype(_np_store())


LAST_RESULTS = None


def kernel(x, w_qkv, w_proj):
    global LAST_RESULTS
    if "nc" not in _cache:
        _cache["nc"] = _build_nc()
    nc = _cache["nc"]

    mask = _make_mask()
    x = np.asarray(x, dtype=np.float32).reshape(B, T, C)
    w_qkv = np.asarray(w_qkv, dtype=np.float32)
    w_proj = np.asarray(w_proj, dtype=np.float32)

    in_maps = []
    for core in range(NCORES):
        b, g = core // 2, core % 2
        fsl = slice(g * F, (g + 1) * F)
        in_maps.append({
            "xT": _prep(x[b].T),
            "wqT": _prep(w_qkv[0 * C:1 * C][fsl].T),
            "wkT": _prep(w_qkv[1 * C:2 * C][fsl].T),
            "wvT": _prep(w_qkv[2 * C:3 * C][fsl].T),
            "woT": _prep(w_proj[:, fsl].T),
            "mask": mask,
            "ones": np.ones((128, 128), _np_store()),
        })

    LAST_RESULTS = run_bass_kernel_spmd(
        nc, in_maps, list(range(NCORES)), trace=checkenv("BASS_TRACE"))

    y = np.zeros((B, T, C), np.float32)
    for core in range(NCORES):
        b = core // 2
        y[b] += LAST_RESULTS.results[core]["yT"].T
    return y


# revision 29
# speedup vs baseline: 1.3439x; 1.3018x over previous
"""Causal self-attention (B=4, T=2048, C=1024, H=16, D=64) on 8 TRN2 NeuronCores.

Sharding: 2D (batch x head-group). Core c handles batch b = c//2 and head
group g = c%2 (heads 8g..8g+7).  Host pre-transposes all inputs so the
device kernel needs no on-chip transposes:
  - xT  [C, T]    : x[b].T
  - wqT/wkT/wvT [C, 512] : w_qkv row-slices for this head group, transposed
  - woT [512, C]  : w_proj column-slice, transposed
Each core computes a partial projected output yT [C, T] for its batch
(contribution of its 8 heads); the host sums the two head-group partials
per batch and transposes back.

Device flow per core (all matmuls are PE `out = lhsT.T @ rhs`, fp32r/TF32):
  A. v = x @ wv^T in natural [tok, feat] layout, resident in SBUF with a
     ones column appended per head (softmax denominator comes free from
     the PV matmul's row 64).
  B. Per head-pair hp: project qT/kT for just these 128 features, then
     attention: scores sT[k,q] via row-tiled concurrent matmuls (two heads
     share the 128-partition dim, K=64 each), exp on ScalarE (1/8 scale
     fused), causal = skip above-diagonal k-tiles + mask diagonal tiles on
     VectorE, PV matmul accumulates oT [65, 512].  The q/k projection of
     hp+1 gives the PE independent work while ACT runs hp's exps (keeps
     the HAM clock-gate at 2.4 GHz).  Normalization: 1/r via exp(-ln(r))
     on ScalarE, partition-broadcast via ones-matmul, multiply on DVE.
  C. yT = woT.T @ o_pack accumulated over the 4 head pairs.
"""

import os
from contextlib import ExitStack

import numpy as np
import ml_dtypes

import concourse.bass as bass
import concourse.bacc as bacc
import concourse.mybir as mybir
import concourse.tile as tile
from concourse.bass_utils import run_bass_kernel_spmd, checkenv

B, T, C = 4, 2048, 1024
H, D = 16, 64
NCORES = 8
F = 512                    # qkv features per matrix per core (8 heads x 64)
CT = C // 128              # 8 contraction tiles
TT = T // 128              # 16 token tiles
QB = T // 512              # 4 query blocks of 512
HP = 4                     # head pairs per core

DTYPE_MODE = os.environ.get("KERNEL_DTYPE", "f32r")  # f32 | f32r | bf16

_F32 = mybir.dt.float32
_EXP = mybir.ActivationFunctionType.Exp
_LN = mybir.ActivationFunctionType.Ln

_cache = {}


def _build_nc():
    dt_store = {"f32": _F32, "f32r": mybir.dt.float32r,
                "bf16": mybir.dt.bfloat16}[DTYPE_MODE]

    nc = bacc.Bacc("TRN2", target_bir_lowering=False, debug=False,
                   num_devices=NCORES)

    xT = nc.dram_tensor("xT", [C, T], dt_store, kind="ExternalInput").ap()
    wqT = nc.dram_tensor("wqT", [C, F], dt_store, kind="ExternalInput").ap()
    wkT = nc.dram_tensor("wkT", [C, F], dt_store, kind="ExternalInput").ap()
    wvT = nc.dram_tensor("wvT", [C, F], dt_store, kind="ExternalInput").ap()
    woT = nc.dram_tensor("woT", [F, C], dt_store, kind="ExternalInput").ap()
    maskd = nc.dram_tensor("mask", [128, 4, 512], dt_store,
                           kind="ExternalInput").ap()
    onesd = nc.dram_tensor("ones", [128, 128], dt_store,
                           kind="ExternalInput").ap()
    yT = nc.dram_tensor("yT", [C, T], _F32, kind="ExternalOutput").ap()

    xT_r = xT.rearrange("(ct p) t -> p ct t", p=128)

    with tile.TileContext(nc) as tc, ExitStack() as top:
        opool = top.enter_context(tc.tile_pool(name="opack", bufs=1))
        onepool = top.enter_context(tc.tile_pool(name="ones", bufs=1))
        ab = top.enter_context(ExitStack())  # pools freed before phase C
        xpool = ab.enter_context(tc.tile_pool(name="xin", bufs=2))
        wqk_pool = ab.enter_context(tc.tile_pool(name="wqk", bufs=1))
        vpool = ab.enter_context(tc.tile_pool(name="vfull", bufs=1))
        mpool = ab.enter_context(tc.tile_pool(name="msk", bufs=1))
        qkpool = ab.enter_context(tc.tile_pool(name="qk", bufs=2))
        epool = ab.enter_context(tc.tile_pool(name="exp", bufs=4))
        osbpool = ab.enter_context(tc.tile_pool(name="osb", bufs=1))
        rcpool = ab.enter_context(tc.tile_pool(name="rc", bufs=1))
        qkvps = ab.enter_context(tc.tile_pool(name="qkv_ps", bufs=2,
                                              space="PSUM"))

        wq_sb = wqk_pool.tile([128, CT, F], dt_store)
        wk_sb = wqk_pool.tile([128, CT, F], dt_store)
        nc.sync.dma_start(out=wq_sb[:],
                          in_=wqT.rearrange("(ct p) f -> p ct f", p=128))
        nc.sync.dma_start(out=wk_sb[:],
                          in_=wkT.rearrange("(ct p) f -> p ct f", p=128))
        mask_sb = mpool.tile([128, 4, 512], dt_store)
        nc.sync.dma_start(out=mask_sb[:], in_=maskd[:])
        ones_sb = onepool.tile([128, 128], dt_store)
        nc.sync.dma_start(out=ones_sb[:], in_=onesd)

        v_full = vpool.tile([128, TT, 8, D + 1], dt_store)
        nc.sync.dma_start(out=v_full[:, :, :, D:D + 1], in_=onesd[:, 0:128])
        o_pack = opool.tile([128, HP, T], dt_store)

        # ---------------- Phase A: v projection (natural layout) --------
        with tc.tile_pool(name="wv", bufs=1) as wvpool:
            wv_sb = wvpool.tile([128, CT, F], dt_store)
            nc.sync.dma_start(out=wv_sb[:],
                              in_=wvT.rearrange("(ct p) f -> p ct f", p=128))
            for tq in range(4):
                x_q = xpool.tile([128, CT, 512], dt_store, tag="x")
                nc.sync.dma_start(out=x_q[:],
                                  in_=xT_r[:, :, tq * 512:(tq + 1) * 512])
                for tl in range(4):
                    ps = qkvps.tile([128, 512], _F32, tag="ps")
                    for ct in range(CT):
                        nc.tensor.matmul(
                            ps[:], x_q[:, ct, tl * 128:(tl + 1) * 128],
                            wv_sb[:, ct, :],
                            start=(ct == 0), stop=(ct == CT - 1))
                    nc.vector.tensor_copy(
                        v_full[:, tq * 4 + tl, :, 0:D],
                        ps[:].rearrange("p (h d) -> p h d", h=8))

        # ---------- Phase B: per head-pair q/k projection + attention ----
        for hp in range(HP):
            fsl = slice(hp * 128, (hp + 1) * 128)
            q_sb = qkpool.tile([128, T], dt_store, tag="q")
            k_sb = qkpool.tile([128, T], dt_store, tag="k")
            for tq in range(4):
                x_q = xpool.tile([128, CT, 512], dt_store, tag="x")
                nc.sync.dma_start(out=x_q[:],
                                  in_=xT_r[:, :, tq * 512:(tq + 1) * 512])
                tsl = slice(tq * 512, (tq + 1) * 512)
                psq = qkvps.tile([128, 512], _F32, tag="ps")
                psk = qkvps.tile([128, 512], _F32, tag="ps")
                for ct in range(CT):
                    nc.tensor.matmul(psq[:], wq_sb[:, ct, fsl],
                                     x_q[:, ct, :],
                                     start=(ct == 0), stop=(ct == CT - 1))
                for ct in range(CT):
                    nc.tensor.matmul(psk[:], wk_sb[:, ct, fsl],
                                     x_q[:, ct, :],
                                     start=(ct == 0), stop=(ct == CT - 1))
                nc.vector.tensor_copy(q_sb[:, tsl], psq[:])
                nc.vector.tensor_copy(k_sb[:, tsl], psk[:])

            with tc.tile_pool(name=f"at{hp}_sc", bufs=4, space="PSUM") as scps, \
                 tc.tile_pool(name=f"at{hp}_o", bufs=2, space="PSUM") as ops:
                for qb in range(QB):
                    kts = 4 * (qb + 1)
                    oA = ops.tile([D + 1, 512], _F32, tag="o")
                    oB = ops.tile([D + 1, 512], _F32, tag="o")
                    qsl = slice(qb * 512, (qb + 1) * 512)

                    pend = []  # software pipeline: scores(kt) ahead of PV(kt-1)

                    def _pv(kt, eA, eB):
                        nc.tensor.matmul(
                            oA[:], v_full[:, kt, 2 * hp, :], eA[:],
                            start=(kt == 0), stop=(kt == kts - 1))
                        nc.tensor.matmul(
                            oB[:], v_full[:, kt, 2 * hp + 1, :], eB[:],
                            start=(kt == 0), stop=(kt == kts - 1))

                    for kt in range(kts):
                        psA = scps.tile([128, 512], _F32, tag="s")
                        psB = scps.tile([128, 512], _F32, tag="s")
                        ksl = slice(kt * 128, (kt + 1) * 128)
                        nc.tensor.matmul(psA[:], k_sb[0:64, ksl],
                                         q_sb[0:64, qsl],
                                         start=True, stop=True,
                                         tile_position=(0, 0))
                        nc.tensor.matmul(psB[:], k_sb[64:128, ksl],
                                         q_sb[64:128, qsl],
                                         start=True, stop=True,
                                         tile_position=(64, 0))
                        eA = epool.tile([128, 512], dt_store, tag="e")
                        eB = epool.tile([128, 512], dt_store, tag="e")
                        nc.scalar.activation(eA[:], psA[:], _EXP, scale=0.125)
                        nc.scalar.activation(eB[:], psB[:], _EXP, scale=0.125)
                        rel = kt - 4 * qb
                        if rel >= 0:  # diagonal tile: causal mask
                            nc.vector.tensor_mul(eA[:], eA[:],
                                                 mask_sb[:, rel, :])
                            nc.vector.tensor_mul(eB[:], eB[:],
                                                 mask_sb[:, rel, :])
                        pend.append((kt, eA, eB))
                        if len(pend) > 1:
                            _pv(*pend.pop(0))
                    _pv(*pend.pop(0))

                    # normalize: o[d, q] * (1/r)[q];  1/r = exp(-ln(r)) on
                    # ScalarE (DVE reciprocal on a 1-partition row is ~6x
                    # slower), then partition-broadcast via ones-matmul.
                    rc = rcpool.tile([128, 1024], _F32, tag="rc")
                    rcx = rcpool.tile([128, 1024], dt_store, tag="rcx")
                    nc.scalar.activation(rc[64:65, 0:512], oA[64:65, :], _LN)
                    nc.scalar.activation(rc[64:65, 512:1024], oB[64:65, :], _LN)
                    nc.scalar.activation(rcx[64:65, 0:512],
                                         rc[64:65, 0:512], _EXP, scale=-1.0)
                    nc.scalar.activation(rcx[64:65, 512:1024],
                                         rc[64:65, 512:1024], _EXP, scale=-1.0)
                    bcA = scps.tile([128, 512], _F32, tag="s")
                    bcB = scps.tile([128, 512], _F32, tag="s")
                    nc.tensor.matmul(bcA[:], ones_sb[64:65, 0:128],
                                     rcx[64:65, 0:512],
                                     start=True, stop=True,
                                     tile_position=(64, 0))
                    nc.tensor.matmul(bcB[:], ones_sb[64:65, 0:128],
                                     rcx[64:65, 512:1024],
                                     start=True, stop=True,
                                     tile_position=(64, 0))
                    # heads live at psum partitions 0-64; head B must land
                    # at o_pack partitions 64-127 -> sbuf->sbuf DMA shift
                    tmp = osbpool.tile([64, 1024], _F32, tag="t")
                    nc.vector.tensor_copy(tmp[0:64, 0:512], oA[0:64, :])
                    nc.vector.tensor_copy(tmp[0:64, 512:1024], oB[0:64, :])
                    ob2 = osbpool.tile([128, 512], _F32, tag="b")
                    nc.sync.dma_start(out=ob2[64:128, :],
                                      in_=tmp[0:64, 512:1024])
                    nc.vector.tensor_mul(o_pack[0:64, hp, qsl],
                                         tmp[0:64, 0:512], bcA[0:64, :])
                    nc.vector.tensor_mul(o_pack[64:128, hp, qsl],
                                         ob2[64:128, :], bcB[64:128, :])

        ab.close()  # free A/B pools (x, weights, v, qk, exp, ...) for phase C

        # ---------------- Phase C: output projection ----------------
        with tc.tile_pool(name="wo", bufs=1) as wopool, \
             tc.tile_pool(name="y_st", bufs=4) as ystpool, \
             tc.tile_pool(name="y_ps", bufs=4, space="PSUM") as yps:
            wo_sb = wopool.tile([128, HP, C], dt_store)
            nc.sync.dma_start(out=wo_sb[:],
                              in_=woT.rearrange("(hp p) o -> p hp o", p=128))
            for ot in range(C // 128):
                for tb in range(QB):
                    ps = yps.tile([128, 512], _F32, tag="y")
                    for hp in range(HP):
                        nc.tensor.matmul(
                            ps[:], wo_sb[:, hp, ot * 128:(ot + 1) * 128],
                            o_pack[:, hp, tb * 512:(tb + 1) * 512],
                            start=(hp == 0), stop=(hp == HP - 1))
                    yst = ystpool.tile([128, 512], _F32, tag="yst")
                    nc.vector.tensor_copy(yst[:], ps[:])
                    nc.sync.dma_start(
                        out=yT[ot * 128:(ot + 1) * 128,
                               tb * 512:(tb + 1) * 512],
                        in_=yst[:])
    # The act-table pass picks the FIRST set containing each function, which
    # makes Exp and Ln thrash between two table sets (~2.7us per swap, per
    # use).  Steer both to the combined natural_log_exp_and_others set by
    # hiding exp/ln from every other set (indices into act_info.json are
    # unchanged, so act_func_set_id stays valid).
    _orig_gat = bacc.get_activation_tables

    def _gat_combined(arch):
        t = _orig_gat(arch)
        for name, fns in t.items():
            if name != "natural_log_exp_and_others":
                fns.discard(mybir.ActivationFunctionType.Exp)
                fns.discard(mybir.ActivationFunctionType.Ln)
        return t

    bacc.get_activation_tables = _gat_combined
    try:
        nc.compile()  # bacc passes: split >1-wait instrs (TRN2 ISA limit)
    finally:
        bacc.get_activation_tables = _orig_gat
    return nc


def _np_store():
    return np.float32 if DTYPE_MODE in ("f32", "f32r") else ml_dtypes.bfloat16


def _round_tf32(a):
    """Round-to-nearest-even onto the TF32 (10-bit mantissa) grid."""
    u = np.ascontiguousarray(a, dtype=np.float32).view(np.uint32)
    r = (u + 0x0FFF + ((u >> 13) & 1)) & np.uint32(0xFFFFE000)
    return r.view(np.float32)


def _prep(a):
    a = np.ascontiguousarray(a, dtype=np.float32)
    if DTYPE_MODE == "f32r":
        return _round_tf32(a)
    return a.astype(_np_store())


def _make_mask():
    kk = np.arange(128)[:, None, None]
    rr = np.arange(4)[None, :, None]
    qq = np.arange(512)[None, None, :]
    return ((rr * 128 + kk) <= qq).astype(_np_store())


LAST_RESULTS = None


def kernel(x, w_qkv, w_proj):
    global LAST_RESULTS
    if "nc" not in _cache:
        _cache["nc"] = _build_nc()
    nc = _cache["nc"]

    mask = _make_mask()
    x = np.asarray(x, dtype=np.float32).reshape(B, T, C)
    w_qkv = np.asarray(w_qkv, dtype=np.float32)
    w_proj = np.asarray(w_proj, dtype=np.float32)

    in_maps = []
    for core in range(NCORES):
        b, g = core // 2, core % 2
        fsl = slice(g * F, (g + 1) * F)
        in_maps.append({
            "xT": _prep(x[b].T),
            "wqT": _prep(w_qkv[0 * C:1 * C][fsl].T),
            "wkT": _prep(w_qkv[1 * C:2 * C][fsl].T),
            "wvT": _prep(w_qkv[2 * C:3 * C][fsl].T),
            "woT": _prep(w_proj[:, fsl].T),
            "mask": mask,
            "ones": np.ones((128, 128), _np_store()),
        })

    LAST_RESULTS = run_bass_kernel_spmd(
        nc, in_maps, list(range(NCORES)), trace=checkenv("BASS_TRACE"))

    y = np.zeros((B, T, C), np.float32)
    for core in range(NCORES):
        b = core // 2
        y[b] += LAST_RESULTS.results[core]["yT"].T
    return y
